# revision 21
# baseline (speedup 1.0000x reference)
"""DimensionalConsistencyLoss on 8 Trainium2 NeuronCores.

The loss touches only gathered rows of the [100000, 512] f32 table: 8192
pos/neg constraints read one row each (sparsity term + target element), 2048
neu constraints read one element. Everything is fetched with row gathers.

Per core (1/8 of the constraints = 1280 slots = 10 columns of 128, dealt by
the host):
  - 10x indirect-DMA row gathers (one [128,512] tile per column; the SWDGE
    Q7 feeds descriptors faster than the 16 SDMA engines drain them, and
    unlike dma_gather this needs no ucode-library load, which costs ~10us).
  - ACT: per tile, activation(Abs, accum_out) -> row |.| sums in one pass.
  - DVE: per tile, scalar_tensor_tensor((ramp == dim_p) * row, accum_out)
    extracts the target element t in one pass.
  - Per-slot coefficient arrays (host-built) unify pos/neg/neu:
        L = w*(Q*|t| + R) + P*|t| + C*rowsum,   w = (S*t >= 0)
  - ones-matmul reduces the [128, COLS] loss matrix to one scalar.

Host sums 8 partial scalars and applies the final scale.
"""

import numpy as np

import concourse.bacc as bacc
import concourse.bass as bass
import concourse.mybir as mybir
from concourse.bass_utils import run_bass_kernel_spmd

P = 128
VOCAB = 100000
DIM = 512
N_POS = 4096
N_NEG = 4096
N_NEU = 2048
N_ALL = N_POS + N_NEG + N_NEU
N_CORES = 8

SLOTS = N_ALL // N_CORES           # 1280
COLS = SLOTS // P                  # 10
RCOLS = (N_POS + N_NEG) // N_CORES // P   # 8 row-gather columns (pos/neg)
# cols RCOLS..COLS-1 are neu: element gathers land t directly in tcol

CONSISTENCY_WEIGHT = 0.5
SPARSITY_WEIGHT = 0.1
C_SP = SPARSITY_WEIGHT / (DIM - 1)

# coefs tensor layout (f32, [128, CW_TOT]): ramp | dims | S | Pp | Q | R | Cc | ones
CW_RAMP = DIM
C_DIMS = CW_RAMP
C_S = C_DIMS + COLS
C_PP = C_S + COLS
C_Q = C_PP + COLS
C_R = C_Q + COLS
C_CC = C_R + COLS
C_ONE = C_CC + COLS
CW_TOT = C_ONE + 1

F32 = mybir.dt.float32
I32 = mybir.dt.int32
AX = mybir.AxisListType.X
OP = mybir.AluOpType
AF = mybir.ActivationFunctionType

_nc_cache = None


def _build_program():
    global _nc_cache
    if _nc_cache is not None:
        return _nc_cache

    nc = bacc.Bacc(
        "TRN2", target_bir_lowering=False, debug=False, num_devices=N_CORES
    )
    emb = nc.dram_tensor("emb", [VOCAB, DIM], F32, kind="ExternalInput")
    idx_d = nc.dram_tensor("idx32", [P, COLS], I32, kind="ExternalInput")
    coef_d = nc.dram_tensor("coefs", [P, CW_TOT], F32, kind="ExternalInput")
    out_d = nc.dram_tensor("out", [P, COLS], F32, kind="ExternalOutput")

    from contextlib import ExitStack

    with ExitStack() as ctx:
        blk_ctx = nc.Block()
        block = blk_ctx.__enter__()
        sb = lambda name, shape, dt=F32: ctx.enter_context(
            nc.sbuf_tensor(name, shape, dt)
        )
        idx_sb = sb("idx_sb", [P, COLS], I32)
        coef_sb = sb("coef_sb", [P, CW_TOT])
        rows = sb("rows", [P, RCOLS, DIM])
        s_act = sb("s_act", [P, RCOLS, DIM])
        s_dve = sb("s_dve", [P, RCOLS, DIM])
        rowsum = sb("rowsum", [P, COLS])
        tcol = sb("tcol", [P, COLS])
        a13 = sb("a13", [P, COLS])
        u13 = sb("u13", [P, COLS])
        w13 = sb("w13", [P, COLS])
        x1 = sb("x1", [P, COLS])
        x2 = sb("x2", [P, COLS])
        x3 = sb("x3", [P, COLS])
        m13 = sb("m13", [P, COLS])
        sem = lambda name: ctx.enter_context(nc.semaphore(name))
        io, io_i, io2 = sem("io"), sem("io_i"), sem("io2")
        gs = [sem(f"gs{j}") for j in range(COLS)]
        dve_x, act_s, dve_f = sem("dve_x"), sem("act_s"), sem("dve_f")
        chain_len = {}
        ramp = coef_sb[:, 0:CW_RAMP]

        @block.gpsimd
        def _(gpsimd: bass.BassGpSimd):
            gpsimd.dma_start(idx_sb[:, :], idx_d[:, :]).then_inc(io_i, 16)
            gpsimd.wait_ge(io_i, 16)
            for j in range(RCOLS):
                gpsimd.indirect_dma_start(
                    out=rows[:, j, :],
                    out_offset=None,
                    in_=emb[:, :],
                    in_offset=bass.IndirectOffsetOnAxis(
                        ap=idx_sb[:, j : j + 1], axis=0
                    ),
                ).then_inc(gs[j], 16)
            for j in range(RCOLS, COLS):
                # neu: flat element gather (idx = id*DIM+dim) lands t directly
                gpsimd.indirect_dma_start(
                    out=tcol[:, j : j + 1],
                    out_offset=None,
                    in_=emb[:, :],
                    in_offset=bass.IndirectOffsetOnAxis(
                        ap=idx_sb[:, j : j + 1], axis=1
                    ),
                ).then_inc(gs[j], 16)

        @block.scalar
        def _(scalar: bass.BassEngine):
            for j in range(RCOLS):
                scalar.wait_ge(gs[j], 16)
                nc.scalar.activation(
                    s_act[:, j, :], rows[:, j, :], AF.Abs,
                    accum_out=rowsum[:, j : j + 1],
                ).then_inc(act_s, 1)

        @block.vector
        def _(vector: bass.BassEngine):
            vector.wait_ge(io, 16)
            for j in range(RCOLS):
                vector.wait_ge(gs[j], 16)
                nc.vector.scalar_tensor_tensor(
                    out=s_dve[:, j, :],
                    in0=ramp,
                    scalar=coef_sb[:, C_DIMS + j : C_DIMS + j + 1],
                    in1=rows[:, j, :],
                    op0=OP.is_equal,
                    op1=OP.mult,
                    accum_out=tcol[:, j : j + 1],
                ).then_inc(dve_x, 1)
            # accum_out writes land late; drain our own pipeline before reads
            vector.wait_ge(dve_x, RCOLS)
            for j in range(RCOLS, COLS):
                vector.wait_ge(gs[j], 16)
            # Same-engine RAW needs explicit sems (deep DVE pipeline).
            # dve_f counts completions; wait on the latest producer.
            # L = w*(Q*a + R) + Pp*a + Cc*rowsum,  w = (t*S>=0), a = |t|
            n = 0

            def step(ins, wait=None):
                nonlocal n
                if wait is not None:
                    vector.wait_ge(dve_f, wait)
                ins().then_inc(dve_f, 1)
                n += 1
                return n

            tS = coef_sb[:, C_S : C_S + COLS]
            i_u = step(lambda: nc.vector.tensor_tensor(
                out=u13[:, :], in0=tcol[:, :], in1=tS, op=OP.mult))
            i_w = step(lambda: nc.vector.tensor_scalar(
                out=w13[:, :], in0=u13[:, :], scalar1=0.0, scalar2=None,
                op0=OP.is_ge), wait=i_u)
            i_m = step(lambda: nc.vector.tensor_scalar(
                out=m13[:, :], in0=w13[:, :], scalar1=2.0, scalar2=-1.0,
                op0=OP.mult, op1=OP.add), wait=i_w)
            i_a = step(lambda: nc.vector.tensor_tensor(
                out=a13[:, :], in0=u13[:, :], in1=m13[:, :], op=OP.mult),
                wait=i_m)
            i1 = step(lambda: nc.vector.tensor_tensor(
                out=x1[:, :], in0=a13[:, :], in1=coef_sb[:, C_Q : C_Q + COLS],
                op=OP.mult), wait=i_a)
            i2 = step(lambda: nc.vector.tensor_tensor(
                out=x2[:, :], in0=a13[:, :], in1=coef_sb[:, C_PP : C_PP + COLS],
                op=OP.mult))
            vector.wait_ge(act_s, RCOLS)
            i3 = step(lambda: nc.vector.tensor_tensor(
                out=x3[:, 0:RCOLS], in0=rowsum[:, 0:RCOLS],
                in1=coef_sb[:, C_CC : C_CC + RCOLS], op=OP.mult))
            i4 = step(lambda: nc.vector.tensor_tensor(
                out=x1[:, :], in0=x1[:, :], in1=coef_sb[:, C_R : C_R + COLS],
                op=OP.add), wait=i1)
            i5 = step(lambda: nc.vector.tensor_tensor(
                out=x1[:, :], in0=x1[:, :], in1=w13[:, :], op=OP.mult), wait=i4)
            i6 = step(lambda: nc.vector.tensor_tensor(
                out=x1[:, :], in0=x1[:, :], in1=x2[:, :], op=OP.add),
                wait=max(i5, i2))
            i7 = step(lambda: nc.vector.tensor_tensor(
                out=x1[:, 0:RCOLS], in0=x1[:, 0:RCOLS], in1=x3[:, 0:RCOLS],
                op=OP.add), wait=max(i6, i3))
            chain_len["n"] = i7

        @block.sync
        def _(sync: bass.BassEngine):
            sync.dma_start(coef_sb[:, :], coef_d[:, :]).then_inc(io, 16)
            sync.wait_ge(dve_f, chain_len["n"])
            sync.dma_start(out_d[:, :], x1[:, :]).then_inc(io2, 16)
            sync.wait_ge(io2, 16)


        blk_ctx.__exit__(None, None, None)
        # The NEFF can be executed repeatedly on one load: clear our
        # semaphores after the end-of-block barrier so every run starts
        # from zero (same dance as Bass.reset()).
        ksr = nc._kernel_sem_range
        mono_start = ksr.start + 3 + (
            1 if nc._bir_kernel_barrier_sem is not None else 0
        )
        user_range = range(mono_start + len(nc._monotonic_sems), ksr.stop)
        nc.gpsimd.dma_reset(user_range)
        nc.gpsimd.sem_clear(user_range)
        nc.all_engine_barrier()

    nc.compile()
    _nc_cache = nc
    return nc


def _deal(pos_ids, pos_dims, neg_ids, neg_dims, neu_ids, neu_dims):
    """Deal all constraints into per-core slot tables (slot j of core c =
    constraint c + 8*j of the concatenated list).

    Returns per-core (idx32 [128, COLS] int32, coefs [128, CW_TOT] f32).
    """
    ids = np.concatenate([pos_ids, neg_ids, neu_ids]).astype(np.int64)
    dims = np.concatenate([pos_dims, neg_dims, neu_dims]).astype(np.int64)
    cls = np.concatenate([
        np.zeros(len(pos_ids), np.int64),
        np.ones(len(neg_ids), np.int64),
        np.full(len(neu_ids), 2, np.int64),
    ])

    idx32 = []
    coefs = []
    for c in range(N_CORES):
        g = np.arange(SLOTS) * N_CORES + c  # this core's constraints
        cid, cdim, ccls = ids[g].copy(), dims[g], cls[g]
        # neu slots gather the element directly: flat index id*DIM+dim
        cid[ccls == 2] = cid[ccls == 2] * DIM + cdim[ccls == 2]
        # slot j -> (p = j%128, col = j//128)
        ix = np.ascontiguousarray(
            cid.reshape(COLS, P).T.astype(np.int32))  # [128, COLS]
        cf = np.zeros((P, CW_TOT), np.float32)
        cf[:, 0:CW_RAMP] = np.arange(DIM, dtype=np.float32)[None, :]
        cf[:, C_ONE] = 1.0
        dm = cdim.reshape(COLS, P).T
        kl = ccls.reshape(COLS, P).T
        cf[:, C_DIMS : C_DIMS + COLS] = dm
        cf[:, C_S : C_S + COLS] = np.where(kl == 0, -1.0, 1.0)
        pn = kl != 2
        cf[:, C_PP : C_PP + COLS] = np.where(
            pn, -SPARSITY_WEIGHT - C_SP, 2.0)
        cf[:, C_Q : C_Q + COLS] = np.where(pn, 1.0 + SPARSITY_WEIGHT, 0.0)
        cf[:, C_R : C_R + COLS] = np.where(pn, SPARSITY_WEIGHT, 0.0)
        cf[:, C_CC : C_CC + COLS] = np.where(pn, C_SP, 0.0)
        idx32.append(ix)
        coefs.append(cf)
    return idx32, coefs


def _make_in_maps(emb, pos_ids, pos_dims, neg_ids, neg_dims, neu_ids, neu_dims):
    idx32, coefs = _deal(pos_ids, pos_dims, neg_ids, neg_dims, neu_ids, neu_dims)
    return [
        {"emb": emb, "idx32": idx32[c], "coefs": coefs[c]}
        for c in range(N_CORES)
    ]


def kernel(**inputs):
    emb = np.ascontiguousarray(np.asarray(inputs["embeddings"], dtype=np.float32))
    ids = {
        k: np.asarray(inputs[k]).astype(np.int64)
        for k in ("pos_ids", "pos_dims", "neg_ids", "neg_dims", "neu_ids", "neu_dims")
    }
    nc = _build_program()
    in_maps = _make_in_maps(
        emb, ids["pos_ids"], ids["pos_dims"], ids["neg_ids"], ids["neg_dims"],
        ids["neu_ids"], ids["neu_dims"],
    )
    res = run_bass_kernel_spmd(nc, in_maps, list(range(N_CORES)))
    total = sum(float(r["out"].astype(np.float64).sum()) for r in res.results)
    val = total * CONSISTENCY_WEIGHT / N_ALL
    return np.asarray(val, dtype=np.float32)


# revision 22
# speedup vs baseline: 1.0593x; 1.0593x over previous
"""DimensionalConsistencyLoss on 8 Trainium2 NeuronCores.

The loss touches only gathered rows of the [100000, 512] f32 table: 8192
pos/neg constraints read one row each (sparsity term + target element), 2048
neu constraints read one element. Everything is fetched with row gathers.

Per core (1/8 of the constraints = 1280 slots = 10 columns of 128, dealt by
the host):
  - 10x indirect-DMA row gathers (one [128,512] tile per column; the SWDGE
    Q7 feeds descriptors faster than the 16 SDMA engines drain them, and
    unlike dma_gather this needs no ucode-library load, which costs ~10us).
  - ACT: per tile, activation(Abs, accum_out) -> row |.| sums in one pass.
  - DVE: per tile, scalar_tensor_tensor((ramp == dim_p) * row, accum_out)
    extracts the target element t in one pass.
  - Per-slot coefficient arrays (host-built) unify pos/neg/neu:
        L = w*(Q*|t| + R) + P*|t| + C*rowsum,   w = (S*t >= 0)
  - ones-matmul reduces the [128, COLS] loss matrix to one scalar.

Host sums 8 partial scalars and applies the final scale.
"""

import numpy as np

import concourse.bacc as bacc
import concourse.bass as bass
import concourse.mybir as mybir
from concourse.bass_utils import run_bass_kernel_spmd

P = 128
VOCAB = 100000
DIM = 512
N_POS = 4096
N_NEG = 4096
N_NEU = 2048
N_ALL = N_POS + N_NEG + N_NEU
N_CORES = 8

SLOTS = N_ALL // N_CORES           # 1280
COLS = SLOTS // P                  # 10
RCOLS = (N_POS + N_NEG) // N_CORES // P   # 8 row-gather columns (pos/neg)
# cols RCOLS..COLS-1 are neu: element gathers land t directly in tcol

CONSISTENCY_WEIGHT = 0.5
SPARSITY_WEIGHT = 0.1
C_SP = SPARSITY_WEIGHT / (DIM - 1)

# coefs tensor layout (f32, [128, CW_TOT]): ramp | dims | S | Pp | Q | R | Cc | ones
CW_RAMP = DIM
C_DIMS = CW_RAMP
C_S = C_DIMS + COLS
C_PP = C_S + COLS
C_Q = C_PP + COLS
C_R = C_Q + COLS
C_CC = C_R + COLS
C_ONE = C_CC + COLS
CW_TOT = C_ONE + 1

F32 = mybir.dt.float32
I32 = mybir.dt.int32
AX = mybir.AxisListType.X
OP = mybir.AluOpType
AF = mybir.ActivationFunctionType

_nc_cache = None


def _build_program():
    global _nc_cache
    if _nc_cache is not None:
        return _nc_cache

    nc = bacc.Bacc(
        "TRN2", target_bir_lowering=False, debug=False, num_devices=N_CORES
    )
    emb = nc.dram_tensor("emb", [VOCAB, DIM], F32, kind="ExternalInput")
    idx_d = nc.dram_tensor("idx32", [P, COLS], I32, kind="ExternalInput")
    coef_d = nc.dram_tensor("coefs", [P, CW_TOT], F32, kind="ExternalInput")
    out_d = nc.dram_tensor("out", [P, COLS], F32, kind="ExternalOutput")

    from contextlib import ExitStack

    with ExitStack() as ctx:
        blk_ctx = nc.Block()
        block = blk_ctx.__enter__()
        sb = lambda name, shape, dt=F32: ctx.enter_context(
            nc.sbuf_tensor(name, shape, dt)
        )
        idx_sb = sb("idx_sb", [P, COLS], I32)
        coef_sb = sb("coef_sb", [P, CW_TOT])
        rows = sb("rows", [P, RCOLS, DIM])
        s_act = sb("s_act", [P, RCOLS, DIM])
        s_dve = sb("s_dve", [P, RCOLS, DIM])
        rowsum = sb("rowsum", [P, COLS])
        tcol = sb("tcol", [P, COLS])
        a13 = sb("a13", [P, COLS])
        u13 = sb("u13", [P, COLS])
        w13 = sb("w13", [P, COLS])
        x1 = sb("x1", [P, COLS])
        x2 = sb("x2", [P, COLS])
        x3 = sb("x3", [P, COLS])
        m13 = sb("m13", [P, COLS])
        sem = lambda name: ctx.enter_context(nc.semaphore(name))
        io, io_i, io2 = sem("io"), sem("io_i"), sem("io2")
        gs = [sem(f"gs{j}") for j in range(COLS)]
        dve_x, act_s, dve_f = sem("dve_x"), sem("act_s"), sem("dve_f")
        chain_len = {}
        ramp = coef_sb[:, 0:CW_RAMP]

        @block.gpsimd
        def _(gpsimd: bass.BassGpSimd):
            gpsimd.wait_ge(io_i, 16)
            for j in range(RCOLS):
                gpsimd.indirect_dma_start(
                    out=rows[:, j, :],
                    out_offset=None,
                    in_=emb[:, :],
                    in_offset=bass.IndirectOffsetOnAxis(
                        ap=idx_sb[:, j : j + 1], axis=0
                    ),
                ).then_inc(gs[j], 16)
            for j in range(RCOLS, COLS):
                # neu: flat element gather (idx = id*DIM+dim) lands t directly
                gpsimd.indirect_dma_start(
                    out=tcol[:, j : j + 1],
                    out_offset=None,
                    in_=emb[:, :],
                    in_offset=bass.IndirectOffsetOnAxis(
                        ap=idx_sb[:, j : j + 1], axis=1
                    ),
                ).then_inc(gs[j], 16)

        @block.scalar
        def _(scalar: bass.BassEngine):
            for j in range(RCOLS):
                scalar.wait_ge(gs[j], 16)
                nc.scalar.activation(
                    s_act[:, j, :], rows[:, j, :], AF.Abs,
                    accum_out=rowsum[:, j : j + 1],
                ).then_inc(act_s, 1)
            scalar.wait_ge(dve_x, RCOLS)
            for j in range(RCOLS, COLS):
                scalar.wait_ge(gs[j], 16)
            nc.scalar.activation(a13[:, :], tcol[:, :], AF.Abs).then_inc(act_s, 1)

        @block.vector
        def _(vector: bass.BassEngine):
            vector.wait_ge(io, 16)
            for j in range(RCOLS):
                vector.wait_ge(gs[j], 16)
                nc.vector.scalar_tensor_tensor(
                    out=s_dve[:, j, :],
                    in0=ramp,
                    scalar=coef_sb[:, C_DIMS + j : C_DIMS + j + 1],
                    in1=rows[:, j, :],
                    op0=OP.is_equal,
                    op1=OP.mult,
                    accum_out=tcol[:, j : j + 1],
                ).then_inc(dve_x, 1)
            # accum_out writes land late; drain our own pipeline before reads
            vector.wait_ge(dve_x, RCOLS)
            for j in range(RCOLS, COLS):
                vector.wait_ge(gs[j], 16)
            # Same-engine RAW needs explicit sems (deep DVE pipeline).
            # dve_f counts completions; wait on the latest producer.
            # L = w*(Q*a + R) + Pp*a + Cc*rowsum,  w = (t*S>=0), a = |t|
            n = 0

            def step(ins, wait=None):
                nonlocal n
                if wait is not None:
                    vector.wait_ge(dve_f, wait)
                ins().then_inc(dve_f, 1)
                n += 1
                return n

            tS = coef_sb[:, C_S : C_S + COLS]
            i_u = step(lambda: nc.vector.tensor_tensor(
                out=u13[:, :], in0=tcol[:, :], in1=tS, op=OP.mult))
            i_w = step(lambda: nc.vector.tensor_scalar(
                out=w13[:, :], in0=u13[:, :], scalar1=0.0, scalar2=None,
                op0=OP.is_ge), wait=i_u)
            vector.wait_ge(act_s, RCOLS + 1)
            i1 = step(lambda: nc.vector.tensor_tensor(
                out=x1[:, :], in0=a13[:, :], in1=coef_sb[:, C_Q : C_Q + COLS],
                op=OP.mult))
            i2 = step(lambda: nc.vector.tensor_tensor(
                out=x2[:, :], in0=a13[:, :], in1=coef_sb[:, C_PP : C_PP + COLS],
                op=OP.mult))
            i3 = step(lambda: nc.vector.tensor_tensor(
                out=x3[:, 0:RCOLS], in0=rowsum[:, 0:RCOLS],
                in1=coef_sb[:, C_CC : C_CC + RCOLS], op=OP.mult))
            i4 = step(lambda: nc.vector.tensor_tensor(
                out=x1[:, :], in0=x1[:, :], in1=coef_sb[:, C_R : C_R + COLS],
                op=OP.add), wait=i1)
            i5 = step(lambda: nc.vector.tensor_tensor(
                out=x1[:, :], in0=x1[:, :], in1=w13[:, :], op=OP.mult), wait=i4)
            i6 = step(lambda: nc.vector.tensor_tensor(
                out=x1[:, :], in0=x1[:, :], in1=x2[:, :], op=OP.add),
                wait=max(i5, i2))
            i7 = step(lambda: nc.vector.tensor_tensor(
                out=x1[:, 0:RCOLS], in0=x1[:, 0:RCOLS], in1=x3[:, 0:RCOLS],
                op=OP.add), wait=max(i6, i3))
            chain_len["n"] = i7

        @block.sync
        def _(sync: bass.BassEngine):
            sync.dma_start(idx_sb[:, :], idx_d[:, :]).then_inc(io_i, 16)
            sync.dma_start(coef_sb[:, :], coef_d[:, :]).then_inc(io, 16)
            sync.wait_ge(dve_f, chain_len["n"])
            sync.dma_start(out_d[:, :], x1[:, :]).then_inc(io2, 16)
            sync.wait_ge(io2, 16)


        blk_ctx.__exit__(None, None, None)
        # The NEFF can be executed repeatedly on one load: clear our
        # semaphores after the end-of-block barrier so every run starts
        # from zero (same dance as Bass.reset()).
        ksr = nc._kernel_sem_range
        mono_start = ksr.start + 3 + (
            1 if nc._bir_kernel_barrier_sem is not None else 0
        )
        user_range = range(mono_start + len(nc._monotonic_sems), ksr.stop)
        nc.gpsimd.dma_reset(user_range)
        nc.gpsimd.sem_clear(user_range)

    nc.compile()
    _nc_cache = nc
    return nc


def _deal(pos_ids, pos_dims, neg_ids, neg_dims, neu_ids, neu_dims):
    """Deal all constraints into per-core slot tables (slot j of core c =
    constraint c + 8*j of the concatenated list).

    Returns per-core (idx32 [128, COLS] int32, coefs [128, CW_TOT] f32).
    """
    ids = np.concatenate([pos_ids, neg_ids, neu_ids]).astype(np.int64)
    dims = np.concatenate([pos_dims, neg_dims, neu_dims]).astype(np.int64)
    cls = np.concatenate([
        np.zeros(len(pos_ids), np.int64),
        np.ones(len(neg_ids), np.int64),
        np.full(len(neu_ids), 2, np.int64),
    ])

    idx32 = []
    coefs = []
    for c in range(N_CORES):
        g = np.arange(SLOTS) * N_CORES + c  # this core's constraints
        cid, cdim, ccls = ids[g].copy(), dims[g], cls[g]
        # neu slots gather the element directly: flat index id*DIM+dim
        cid[ccls == 2] = cid[ccls == 2] * DIM + cdim[ccls == 2]
        # slot j -> (p = j%128, col = j//128)
        ix = np.ascontiguousarray(
            cid.reshape(COLS, P).T.astype(np.int32))  # [128, COLS]
        cf = np.zeros((P, CW_TOT), np.float32)
        cf[:, 0:CW_RAMP] = np.arange(DIM, dtype=np.float32)[None, :]
        cf[:, C_ONE] = 1.0
        dm = cdim.reshape(COLS, P).T
        kl = ccls.reshape(COLS, P).T
        cf[:, C_DIMS : C_DIMS + COLS] = dm
        cf[:, C_S : C_S + COLS] = np.where(kl == 0, -1.0, 1.0)
        pn = kl != 2
        cf[:, C_PP : C_PP + COLS] = np.where(
            pn, -SPARSITY_WEIGHT - C_SP, 2.0)
        cf[:, C_Q : C_Q + COLS] = np.where(pn, 1.0 + SPARSITY_WEIGHT, 0.0)
        cf[:, C_R : C_R + COLS] = np.where(pn, SPARSITY_WEIGHT, 0.0)
        cf[:, C_CC : C_CC + COLS] = np.where(pn, C_SP, 0.0)
        idx32.append(ix)
        coefs.append(cf)
    return idx32, coefs


def _make_in_maps(emb, pos_ids, pos_dims, neg_ids, neg_dims, neu_ids, neu_dims):
    idx32, coefs = _deal(pos_ids, pos_dims, neg_ids, neg_dims, neu_ids, neu_dims)
    return [
        {"emb": emb, "idx32": idx32[c], "coefs": coefs[c]}
        for c in range(N_CORES)
    ]


def kernel(**inputs):
    emb = np.ascontiguousarray(np.asarray(inputs["embeddings"], dtype=np.float32))
    ids = {
        k: np.asarray(inputs[k]).astype(np.int64)
        for k in ("pos_ids", "pos_dims", "neg_ids", "neg_dims", "neu_ids", "neu_dims")
    }
    nc = _build_program()
    in_maps = _make_in_maps(
        emb, ids["pos_ids"], ids["pos_dims"], ids["neg_ids"], ids["neg_dims"],
        ids["neu_ids"], ids["neu_dims"],
    )
    res = run_bass_kernel_spmd(nc, in_maps, list(range(N_CORES)))
    total = sum(float(r["out"].astype(np.float64).sum()) for r in res.results)
    val = total * CONSISTENCY_WEIGHT / N_ALL
    return np.asarray(val, dtype=np.float32)


# revision 23
# speedup vs baseline: 1.0663x; 1.0065x over previous
"""DimensionalConsistencyLoss on 8 Trainium2 NeuronCores.

The loss touches only gathered rows of the [100000, 512] f32 table: 8192
pos/neg constraints read one row each (sparsity term + target element), 2048
neu constraints read one element. Everything is fetched with row gathers.

Per core (1/8 of the constraints = 1280 slots = 10 columns of 128, dealt by
the host):
  - 10x indirect-DMA row gathers (one [128,512] tile per column; the SWDGE
    Q7 feeds descriptors faster than the 16 SDMA engines drain them, and
    unlike dma_gather this needs no ucode-library load, which costs ~10us).
  - ACT: per tile, activation(Abs, accum_out) -> row |.| sums in one pass.
  - DVE: per tile, scalar_tensor_tensor((ramp == dim_p) * row, accum_out)
    extracts the target element t in one pass.
  - Per-slot coefficient arrays (host-built) unify pos/neg/neu:
        L = w*(Q*|t| + R) + P*|t| + C*rowsum,   w = (S*t >= 0)
  - ones-matmul reduces the [128, COLS] loss matrix to one scalar.

Host sums 8 partial scalars and applies the final scale.
"""

import numpy as np

import concourse.bacc as bacc
import concourse.bass as bass
import concourse.mybir as mybir
from concourse.bass_utils import run_bass_kernel_spmd

P = 128
VOCAB = 100000
DIM = 512
N_POS = 4096
N_NEG = 4096
N_NEU = 2048
N_ALL = N_POS + N_NEG + N_NEU
N_CORES = 8

SLOTS = N_ALL // N_CORES           # 1280
COLS = SLOTS // P                  # 10
RCOLS = (N_POS + N_NEG) // N_CORES // P   # 8 row-gather columns (pos/neg)
# cols RCOLS..COLS-1 are neu: element gathers land t directly in tcol

CONSISTENCY_WEIGHT = 0.5
SPARSITY_WEIGHT = 0.1
C_SP = SPARSITY_WEIGHT / (DIM - 1)

# coefs tensor layout (f32, [128, CW_TOT]): ramp | dims | S | Pp | Q | R | Cc | ones
CW_RAMP = DIM
C_DIMS = CW_RAMP
C_S = C_DIMS + COLS
C_PP = C_S + COLS
C_Q = C_PP + COLS
C_R = C_Q + COLS
C_CC = C_R + COLS
C_ONE = C_CC + COLS
CW_TOT = C_ONE + 1

F32 = mybir.dt.float32
I32 = mybir.dt.int32
AX = mybir.AxisListType.X
OP = mybir.AluOpType
AF = mybir.ActivationFunctionType

_nc_cache = None


def _build_program():
    global _nc_cache
    if _nc_cache is not None:
        return _nc_cache

    nc = bacc.Bacc(
        "TRN2", target_bir_lowering=False, debug=False, num_devices=N_CORES,
        num_swdge_queues=4,
    )
    emb = nc.dram_tensor("emb", [VOCAB, DIM], F32, kind="ExternalInput")
    idx_d = nc.dram_tensor("idx32", [P, COLS], I32, kind="ExternalInput")
    coef_d = nc.dram_tensor("coefs", [P, CW_TOT], F32, kind="ExternalInput")
    out_d = nc.dram_tensor("out", [P, COLS], F32, kind="ExternalOutput")

    from contextlib import ExitStack

    with ExitStack() as ctx:
        blk_ctx = nc.Block()
        block = blk_ctx.__enter__()
        sb = lambda name, shape, dt=F32: ctx.enter_context(
            nc.sbuf_tensor(name, shape, dt)
        )
        idx_sb = sb("idx_sb", [P, COLS], I32)
        coef_sb = sb("coef_sb", [P, CW_TOT])
        rows = sb("rows", [P, RCOLS, DIM])
        s_act = sb("s_act", [P, RCOLS, DIM])
        s_dve = sb("s_dve", [P, RCOLS, DIM])
        rowsum = sb("rowsum", [P, COLS])
        tcol = sb("tcol", [P, COLS])
        a13 = sb("a13", [P, COLS])
        u13 = sb("u13", [P, COLS])
        w13 = sb("w13", [P, COLS])
        x1 = sb("x1", [P, COLS])
        x2 = sb("x2", [P, COLS])
        x3 = sb("x3", [P, COLS])
        m13 = sb("m13", [P, COLS])
        sem = lambda name: ctx.enter_context(nc.semaphore(name))
        io, io_i, io2 = sem("io"), sem("io_i"), sem("io2")
        gs = [sem(f"gs{j}") for j in range(COLS)]
        dve_x, act_s, dve_f = sem("dve_x"), sem("act_s"), sem("dve_f")
        chain_len = {}
        ramp = coef_sb[:, 0:CW_RAMP]

        @block.gpsimd
        def _(gpsimd: bass.BassGpSimd):
            gpsimd.wait_ge(io_i, 16)
            # Stripe gathers across the 4 SWDGE queues -> 4 Q7 cpu pairs
            # generate descriptors in parallel.
            for j in range(RCOLS):
                inst = gpsimd.indirect_dma_start(
                    out=rows[:, j, :],
                    out_offset=None,
                    in_=emb[:, :],
                    in_offset=bass.IndirectOffsetOnAxis(
                        ap=idx_sb[:, j : j + 1], axis=0
                    ),
                ).then_inc(gs[j], 16)
                inst.ins.queue = f"qPoolDynamic{j % 4 or ''}"
            for j in range(RCOLS, COLS):
                # neu: flat element gather (idx = id*DIM+dim) lands t directly
                inst = gpsimd.indirect_dma_start(
                    out=tcol[:, j : j + 1],
                    out_offset=None,
                    in_=emb[:, :],
                    in_offset=bass.IndirectOffsetOnAxis(
                        ap=idx_sb[:, j : j + 1], axis=1
                    ),
                ).then_inc(gs[j], 16)
                inst.ins.queue = f"qPoolDynamic{j % 4 or ''}"

        @block.scalar
        def _(scalar: bass.BassEngine):
            for j in range(RCOLS):
                scalar.wait_ge(gs[j], 16)
                nc.scalar.activation(
                    s_act[:, j, :], rows[:, j, :], AF.Abs,
                    accum_out=rowsum[:, j : j + 1],
                ).then_inc(act_s, 1)
            scalar.wait_ge(dve_x, RCOLS)
            for j in range(RCOLS, COLS):
                scalar.wait_ge(gs[j], 16)
            nc.scalar.activation(a13[:, :], tcol[:, :], AF.Abs).then_inc(act_s, 1)

        @block.vector
        def _(vector: bass.BassEngine):
            vector.wait_ge(io, 16)
            for j in range(RCOLS):
                vector.wait_ge(gs[j], 16)
                nc.vector.scalar_tensor_tensor(
                    out=s_dve[:, j, :],
                    in0=ramp,
                    scalar=coef_sb[:, C_DIMS + j : C_DIMS + j + 1],
                    in1=rows[:, j, :],
                    op0=OP.is_equal,
                    op1=OP.mult,
                    accum_out=tcol[:, j : j + 1],
                ).then_inc(dve_x, 1)
            # accum_out writes land late; drain our own pipeline before reads
            vector.wait_ge(dve_x, RCOLS)
            for j in range(RCOLS, COLS):
                vector.wait_ge(gs[j], 16)
            # Same-engine RAW needs explicit sems (deep DVE pipeline).
            # dve_f counts completions; wait on the latest producer.
            # L = w*(Q*a + R) + Pp*a + Cc*rowsum,  w = (t*S>=0), a = |t|
            n = 0

            def step(ins, wait=None):
                nonlocal n
                if wait is not None:
                    vector.wait_ge(dve_f, wait)
                ins().then_inc(dve_f, 1)
                n += 1
                return n

            tS = coef_sb[:, C_S : C_S + COLS]
            i_u = step(lambda: nc.vector.tensor_tensor(
                out=u13[:, :], in0=tcol[:, :], in1=tS, op=OP.mult))
            i_w = step(lambda: nc.vector.tensor_scalar(
                out=w13[:, :], in0=u13[:, :], scalar1=0.0, scalar2=None,
                op0=OP.is_ge), wait=i_u)
            vector.wait_ge(act_s, RCOLS + 1)
            i1 = step(lambda: nc.vector.tensor_tensor(
                out=x1[:, :], in0=a13[:, :], in1=coef_sb[:, C_Q : C_Q + COLS],
                op=OP.mult))
            i2 = step(lambda: nc.vector.tensor_tensor(
                out=x2[:, :], in0=a13[:, :], in1=coef_sb[:, C_PP : C_PP + COLS],
                op=OP.mult))
            i3 = step(lambda: nc.vector.tensor_tensor(
                out=x3[:, 0:RCOLS], in0=rowsum[:, 0:RCOLS],
                in1=coef_sb[:, C_CC : C_CC + RCOLS], op=OP.mult))
            i4 = step(lambda: nc.vector.tensor_tensor(
                out=x1[:, :], in0=x1[:, :], in1=coef_sb[:, C_R : C_R + COLS],
                op=OP.add), wait=i1)
            i5 = step(lambda: nc.vector.tensor_tensor(
                out=x1[:, :], in0=x1[:, :], in1=w13[:, :], op=OP.mult), wait=i4)
            i6 = step(lambda: nc.vector.tensor_tensor(
                out=x1[:, :], in0=x1[:, :], in1=x2[:, :], op=OP.add),
                wait=max(i5, i2))
            i7 = step(lambda: nc.vector.tensor_tensor(
                out=x1[:, 0:RCOLS], in0=x1[:, 0:RCOLS], in1=x3[:, 0:RCOLS],
                op=OP.add), wait=max(i6, i3))
            chain_len["n"] = i7

        @block.sync
        def _(sync: bass.BassEngine):
            sync.dma_start(idx_sb[:, :], idx_d[:, :]).then_inc(io_i, 16)
            sync.dma_start(coef_sb[:, :], coef_d[:, :]).then_inc(io, 16)
            sync.wait_ge(dve_f, chain_len["n"])
            sync.dma_start(out_d[:, :], x1[:, :]).then_inc(io2, 16)
            sync.wait_ge(io2, 16)


        blk_ctx.__exit__(None, None, None)
        # The NEFF can be executed repeatedly on one load: clear our
        # semaphores after the end-of-block barrier so every run starts
        # from zero (same dance as Bass.reset()).
        ksr = nc._kernel_sem_range
        mono_start = ksr.start + 3 + (
            1 if nc._bir_kernel_barrier_sem is not None else 0
        )
        user_range = range(mono_start + len(nc._monotonic_sems), ksr.stop)
        nc.gpsimd.dma_reset(user_range)
        nc.gpsimd.sem_clear(user_range)

    nc.compile()
    _nc_cache = nc
    return nc


def _deal(pos_ids, pos_dims, neg_ids, neg_dims, neu_ids, neu_dims):
    """Deal all constraints into per-core slot tables (slot j of core c =
    constraint c + 8*j of the concatenated list).

    Returns per-core (idx32 [128, COLS] int32, coefs [128, CW_TOT] f32).
    """
    ids = np.concatenate([pos_ids, neg_ids, neu_ids]).astype(np.int64)
    dims = np.concatenate([pos_dims, neg_dims, neu_dims]).astype(np.int64)
    cls = np.concatenate([
        np.zeros(len(pos_ids), np.int64),
        np.ones(len(neg_ids), np.int64),
        np.full(len(neu_ids), 2, np.int64),
    ])

    idx32 = []
    coefs = []
    for c in range(N_CORES):
        g = np.arange(SLOTS) * N_CORES + c  # this core's constraints
        cid, cdim, ccls = ids[g].copy(), dims[g], cls[g]
        # neu slots gather the element directly: flat index id*DIM+dim
        cid[ccls == 2] = cid[ccls == 2] * DIM + cdim[ccls == 2]
        # slot j -> (p = j%128, col = j//128)
        ix = np.ascontiguousarray(
            cid.reshape(COLS, P).T.astype(np.int32))  # [128, COLS]
        cf = np.zeros((P, CW_TOT), np.float32)
        cf[:, 0:CW_RAMP] = np.arange(DIM, dtype=np.float32)[None, :]
        cf[:, C_ONE] = 1.0
        dm = cdim.reshape(COLS, P).T
        kl = ccls.reshape(COLS, P).T
        cf[:, C_DIMS : C_DIMS + COLS] = dm
        cf[:, C_S : C_S + COLS] = np.where(kl == 0, -1.0, 1.0)
        pn = kl != 2
        cf[:, C_PP : C_PP + COLS] = np.where(
            pn, -SPARSITY_WEIGHT - C_SP, 2.0)
        cf[:, C_Q : C_Q + COLS] = np.where(pn, 1.0 + SPARSITY_WEIGHT, 0.0)
        cf[:, C_R : C_R + COLS] = np.where(pn, SPARSITY_WEIGHT, 0.0)
        cf[:, C_CC : C_CC + COLS] = np.where(pn, C_SP, 0.0)
        idx32.append(ix)
        coefs.append(cf)
    return idx32, coefs


def _make_in_maps(emb, pos_ids, pos_dims, neg_ids, neg_dims, neu_ids, neu_dims):
    idx32, coefs = _deal(pos_ids, pos_dims, neg_ids, neg_dims, neu_ids, neu_dims)
    return [
        {"emb": emb, "idx32": idx32[c], "coefs": coefs[c]}
        for c in range(N_CORES)
    ]


def kernel(**inputs):
    emb = np.ascontiguousarray(np.asarray(inputs["embeddings"], dtype=np.float32))
    ids = {
        k: np.asarray(inputs[k]).astype(np.int64)
        for k in ("pos_ids", "pos_dims", "neg_ids", "neg_dims", "neu_ids", "neu_dims")
    }
    nc = _build_program()
    in_maps = _make_in_maps(
        emb, ids["pos_ids"], ids["pos_dims"], ids["neg_ids"], ids["neg_dims"],
        ids["neu_ids"], ids["neu_dims"],
    )
    res = run_bass_kernel_spmd(nc, in_maps, list(range(N_CORES)))
    total = sum(float(r["out"].astype(np.float64).sum()) for r in res.results)
    val = total * CONSISTENCY_WEIGHT / N_ALL
    return np.asarray(val, dtype=np.float32)


# revision 24
# speedup vs baseline: 1.0888x; 1.0211x over previous
"""DimensionalConsistencyLoss on 8 Trainium2 NeuronCores.

The loss touches only gathered rows of the [100000, 512] f32 table: 8192
pos/neg constraints read one row each (sparsity term + target element), 2048
neu constraints read one element. Everything is fetched with row gathers.

Per core (1/8 of the constraints = 1280 slots = 10 columns of 128, dealt by
the host):
  - 10x indirect-DMA row gathers (one [128,512] tile per column; the SWDGE
    Q7 feeds descriptors faster than the 16 SDMA engines drain them, and
    unlike dma_gather this needs no ucode-library load, which costs ~10us).
  - ACT: per tile, activation(Abs, accum_out) -> row |.| sums in one pass.
  - DVE: per tile, scalar_tensor_tensor((ramp == dim_p) * row, accum_out)
    extracts the target element t in one pass.
  - Per-slot coefficient arrays (host-built) unify pos/neg/neu:
        L = w*(Q*|t| + R) + P*|t| + C*rowsum,   w = (S*t >= 0)
  - ones-matmul reduces the [128, COLS] loss matrix to one scalar.

Host sums 8 partial scalars and applies the final scale.
"""

import numpy as np

import concourse.bacc as bacc
import concourse.bass as bass
import concourse.mybir as mybir
from concourse.bass_utils import run_bass_kernel_spmd

P = 128
VOCAB = 100000
DIM = 512
N_POS = 4096
N_NEG = 4096
N_NEU = 2048
N_ALL = N_POS + N_NEG + N_NEU
N_CORES = 8

SLOTS = N_ALL // N_CORES           # 1280
COLS = SLOTS // P                  # 10
RCOLS = (N_POS + N_NEG) // N_CORES // P   # 8 row-gather columns (pos/neg)
# cols RCOLS..COLS-1 are neu: element gathers land t directly in tcol

CONSISTENCY_WEIGHT = 0.5
SPARSITY_WEIGHT = 0.1
C_SP = SPARSITY_WEIGHT / (DIM - 1)

# coefs tensor layout (f32, [128, CW_TOT]): ramp | dims | S | Pp | Q | R | Cc | ones
CW_RAMP = DIM
C_DIMS = CW_RAMP
C_S = C_DIMS + COLS
C_PP = C_S + COLS
C_Q = C_PP + COLS
C_R = C_Q + COLS
C_CC = C_R + COLS
C_ONE = C_CC + COLS
CW_TOT = C_ONE + 1

F32 = mybir.dt.float32
I32 = mybir.dt.int32
AX = mybir.AxisListType.X
OP = mybir.AluOpType
AF = mybir.ActivationFunctionType

_nc_cache = None


def _build_program():
    global _nc_cache
    if _nc_cache is not None:
        return _nc_cache

    nc = bacc.Bacc(
        "TRN2", target_bir_lowering=False, debug=False, num_devices=N_CORES,
        num_swdge_queues=4,
    )
    emb = nc.dram_tensor("emb", [VOCAB, DIM], F32, kind="ExternalInput")
    idx_d = nc.dram_tensor("idx32", [P, COLS], I32, kind="ExternalInput")
    coef_d = nc.dram_tensor("coefs", [P, CW_TOT], F32, kind="ExternalInput")
    out_d = nc.dram_tensor("out", [P, COLS], F32, kind="ExternalOutput")

    from contextlib import ExitStack

    with ExitStack() as ctx:
        sb = lambda name, shape, dt=F32: ctx.enter_context(
            nc.sbuf_tensor(name, shape, dt)
        )
        idx_sb = sb("idx_sb", [P, COLS], I32)
        coef_sb = sb("coef_sb", [P, CW_TOT])
        rows = sb("rows", [P, RCOLS, DIM])
        s_act = sb("s_act", [P, RCOLS, DIM])
        s_dve = sb("s_dve", [P, RCOLS, DIM])
        rowsum = sb("rowsum", [P, COLS])
        tcol = sb("tcol", [P, COLS])
        a13 = sb("a13", [P, COLS])
        u13 = sb("u13", [P, COLS])
        w13 = sb("w13", [P, COLS])
        x1 = sb("x1", [P, COLS])
        x2 = sb("x2", [P, COLS])
        x3 = sb("x3", [P, COLS])
        m13 = sb("m13", [P, COLS])
        sem = lambda name: ctx.enter_context(nc.semaphore(name))
        io, io_i, io2 = sem("io"), sem("io_i"), sem("io2")
        gs = [sem(f"gs{j}") for j in range(COLS)]
        dve_x, act_s, dve_f = sem("dve_x"), sem("act_s"), sem("dve_f")
        chain_len = {}
        ramp = coef_sb[:, 0:CW_RAMP]

        # Issue input loads before the Block so they overlap its entry.
        nc.sync.dma_start(idx_sb[:, :], idx_d[:, :]).then_inc(io_i, 16)
        nc.sync.dma_start(coef_sb[:, :], coef_d[:, :]).then_inc(io, 16)

        blk_ctx = nc.Block()
        block = blk_ctx.__enter__()

        @block.gpsimd
        def _(gpsimd: bass.BassGpSimd):
            gpsimd.wait_ge(io_i, 16)
            # Stripe gathers across the 4 SWDGE queues -> 4 Q7 cpu pairs
            # generate descriptors in parallel.
            for j in range(RCOLS):
                inst = gpsimd.indirect_dma_start(
                    out=rows[:, j, :],
                    out_offset=None,
                    in_=emb[:, :],
                    in_offset=bass.IndirectOffsetOnAxis(
                        ap=idx_sb[:, j : j + 1], axis=0
                    ),
                ).then_inc(gs[j], 16)
                inst.ins.queue = f"qPoolDynamic{j % 4 or ''}"
            for j in range(RCOLS, COLS):
                # neu: flat element gather (idx = id*DIM+dim) lands t directly
                inst = gpsimd.indirect_dma_start(
                    out=tcol[:, j : j + 1],
                    out_offset=None,
                    in_=emb[:, :],
                    in_offset=bass.IndirectOffsetOnAxis(
                        ap=idx_sb[:, j : j + 1], axis=1
                    ),
                ).then_inc(gs[j], 16)
                inst.ins.queue = f"qPoolDynamic{j % 4 or ''}"

        @block.scalar
        def _(scalar: bass.BassEngine):
            for j in range(RCOLS):
                scalar.wait_ge(gs[j], 16)
                nc.scalar.activation(
                    s_act[:, j, :], rows[:, j, :], AF.Abs,
                    accum_out=rowsum[:, j : j + 1],
                ).then_inc(act_s, 1)
            scalar.wait_ge(dve_x, RCOLS)
            for j in range(RCOLS, COLS):
                scalar.wait_ge(gs[j], 16)
            nc.scalar.activation(a13[:, :], tcol[:, :], AF.Abs).then_inc(act_s, 1)

        @block.vector
        def _(vector: bass.BassEngine):
            vector.wait_ge(io, 16)
            for j in range(RCOLS):
                vector.wait_ge(gs[j], 16)
                nc.vector.scalar_tensor_tensor(
                    out=s_dve[:, j, :],
                    in0=ramp,
                    scalar=coef_sb[:, C_DIMS + j : C_DIMS + j + 1],
                    in1=rows[:, j, :],
                    op0=OP.is_equal,
                    op1=OP.mult,
                    accum_out=tcol[:, j : j + 1],
                ).then_inc(dve_x, 1)
            # accum_out writes land late; drain our own pipeline before reads
            vector.wait_ge(dve_x, RCOLS)
            for j in range(RCOLS, COLS):
                vector.wait_ge(gs[j], 16)
            # Same-engine RAW needs explicit sems (deep DVE pipeline).
            # dve_f counts completions; wait on the latest producer.
            # L = w*(Q*a + R) + Pp*a + Cc*rowsum,  w = (t*S>=0), a = |t|
            n = 0

            def step(ins, wait=None):
                nonlocal n
                if wait is not None:
                    vector.wait_ge(dve_f, wait)
                ins().then_inc(dve_f, 1)
                n += 1
                return n

            tS = coef_sb[:, C_S : C_S + COLS]
            i_u = step(lambda: nc.vector.tensor_tensor(
                out=u13[:, :], in0=tcol[:, :], in1=tS, op=OP.mult))
            i_w = step(lambda: nc.vector.tensor_scalar(
                out=w13[:, :], in0=u13[:, :], scalar1=0.0, scalar2=None,
                op0=OP.is_ge), wait=i_u)
            vector.wait_ge(act_s, RCOLS + 1)
            i1 = step(lambda: nc.vector.tensor_tensor(
                out=x1[:, :], in0=a13[:, :], in1=coef_sb[:, C_Q : C_Q + COLS],
                op=OP.mult))
            i2 = step(lambda: nc.vector.tensor_tensor(
                out=x2[:, :], in0=a13[:, :], in1=coef_sb[:, C_PP : C_PP + COLS],
                op=OP.mult))
            i3 = step(lambda: nc.vector.tensor_tensor(
                out=x3[:, 0:RCOLS], in0=rowsum[:, 0:RCOLS],
                in1=coef_sb[:, C_CC : C_CC + RCOLS], op=OP.mult))
            i4 = step(lambda: nc.vector.tensor_tensor(
                out=x1[:, :], in0=x1[:, :], in1=coef_sb[:, C_R : C_R + COLS],
                op=OP.add), wait=i1)
            i5 = step(lambda: nc.vector.tensor_tensor(
                out=x1[:, :], in0=x1[:, :], in1=w13[:, :], op=OP.mult), wait=i4)
            i6 = step(lambda: nc.vector.tensor_tensor(
                out=x1[:, :], in0=x1[:, :], in1=x2[:, :], op=OP.add),
                wait=max(i5, i2))
            i7 = step(lambda: nc.vector.tensor_tensor(
                out=x1[:, 0:RCOLS], in0=x1[:, 0:RCOLS], in1=x3[:, 0:RCOLS],
                op=OP.add), wait=max(i6, i3))
            chain_len["n"] = i7

        @block.sync
        def _(sync: bass.BassEngine):
            sync.wait_ge(dve_f, chain_len["n"])
            sync.dma_start(out_d[:, :], x1[:, :]).then_inc(io2, 16)
            sync.wait_ge(io2, 16)


        blk_ctx.__exit__(None, None, None)
        # The NEFF can be executed repeatedly on one load: clear our
        # semaphores after the end-of-block barrier so every run starts
        # from zero (same dance as Bass.reset()).
        ksr = nc._kernel_sem_range
        mono_start = ksr.start + 3 + (
            1 if nc._bir_kernel_barrier_sem is not None else 0
        )
        user_range = range(mono_start + len(nc._monotonic_sems), ksr.stop)
        nc.gpsimd.sem_clear(user_range)

    nc.compile()
    _nc_cache = nc
    return nc


def _deal(pos_ids, pos_dims, neg_ids, neg_dims, neu_ids, neu_dims):
    """Deal all constraints into per-core slot tables (slot j of core c =
    constraint c + 8*j of the concatenated list).

    Returns per-core (idx32 [128, COLS] int32, coefs [128, CW_TOT] f32).
    """
    ids = np.concatenate([pos_ids, neg_ids, neu_ids]).astype(np.int64)
    dims = np.concatenate([pos_dims, neg_dims, neu_dims]).astype(np.int64)
    cls = np.concatenate([
        np.zeros(len(pos_ids), np.int64),
        np.ones(len(neg_ids), np.int64),
        np.full(len(neu_ids), 2, np.int64),
    ])

    idx32 = []
    coefs = []
    for c in range(N_CORES):
        g = np.arange(SLOTS) * N_CORES + c  # this core's constraints
        cid, cdim, ccls = ids[g].copy(), dims[g], cls[g]
        # neu slots gather the element directly: flat index id*DIM+dim
        cid[ccls == 2] = cid[ccls == 2] * DIM + cdim[ccls == 2]
        # slot j -> (p = j%128, col = j//128)
        ix = np.ascontiguousarray(
            cid.reshape(COLS, P).T.astype(np.int32))  # [128, COLS]
        cf = np.zeros((P, CW_TOT), np.float32)
        cf[:, 0:CW_RAMP] = np.arange(DIM, dtype=np.float32)[None, :]
        cf[:, C_ONE] = 1.0
        dm = cdim.reshape(COLS, P).T
        kl = ccls.reshape(COLS, P).T
        cf[:, C_DIMS : C_DIMS + COLS] = dm
        cf[:, C_S : C_S + COLS] = np.where(kl == 0, -1.0, 1.0)
        pn = kl != 2
        cf[:, C_PP : C_PP + COLS] = np.where(
            pn, -SPARSITY_WEIGHT - C_SP, 2.0)
        cf[:, C_Q : C_Q + COLS] = np.where(pn, 1.0 + SPARSITY_WEIGHT, 0.0)
        cf[:, C_R : C_R + COLS] = np.where(pn, SPARSITY_WEIGHT, 0.0)
        cf[:, C_CC : C_CC + COLS] = np.where(pn, C_SP, 0.0)
        idx32.append(ix)
        coefs.append(cf)
    return idx32, coefs


def _make_in_maps(emb, pos_ids, pos_dims, neg_ids, neg_dims, neu_ids, neu_dims):
    idx32, coefs = _deal(pos_ids, pos_dims, neg_ids, neg_dims, neu_ids, neu_dims)
    return [
        {"emb": emb, "idx32": idx32[c], "coefs": coefs[c]}
        for c in range(N_CORES)
    ]


def kernel(**inputs):
    emb = np.ascontiguousarray(np.asarray(inputs["embeddings"], dtype=np.float32))
    ids = {
        k: np.asarray(inputs[k]).astype(np.int64)
        for k in ("pos_ids", "pos_dims", "neg_ids", "neg_dims", "neu_ids", "neu_dims")
    }
    nc = _build_program()
    in_maps = _make_in_maps(
        emb, ids["pos_ids"], ids["pos_dims"], ids["neg_ids"], ids["neg_dims"],
        ids["neu_ids"], ids["neu_dims"],
    )
    res = run_bass_kernel_spmd(nc, in_maps, list(range(N_CORES)))
    total = sum(float(r["out"].astype(np.float64).sum()) for r in res.results)
    val = total * CONSISTENCY_WEIGHT / N_ALL
    return np.asarray(val, dtype=np.float32)
